# revision 5
# baseline (speedup 1.0000x reference)
"""Chamfer loss on 8 trn2 NeuronCores (Bass/Tile).

Reference computation (per batch b):
    d2[n, m] = ||pred[b,n] - target[b,m]||^2   (floored at 0)
    loss = mean_n min_m d2 + mean_m min_n d2,  averaged over batches.

Strategy (matches the data-parallel + N-tiling sharding hint):
  - 8 cores = 4 batches x 2 halves of N (rows of pred).
  - Core (b, h) computes nd2 = -d2 for its [4096 x 8192] block via a single
    K=5 augmented matmul on the PE:
        nd2[n, m] = 2 p.t - |p|^2 - |t|^2
    with lhsT = (2px, 2py, 2pz, |p|^2, 1), rhs = (tx, ty, tz, -1, -|t|^2),
    both fed as float32r (full-speed PE at free dim 512).
  - Row path (cham_x): running free-dim max of nd2 via fused
    tensor_tensor_reduce on DVE (bf16, after an ACT cast of PSUM->SBUF).
    Complete per core: no cross-core reduction.
  - Col path (cham_y partial): elementwise max accumulator [128, 8192] bf16
    over the 32 row tiles on DVE; final partition-axis fold via PE transposes
    + DVE reduces. Host min-reduces the two halves of each batch.
  - Host: tiny [128,32]+[128,64] outputs per core -> final scalar in numpy.
"""

import numpy as np
import ml_dtypes

B = 4
NPTS = 8192          # pred points per batch
MPTS = 8192          # target points per batch
NH = NPTS // 2       # rows per core
P = 128              # partitions
FD = 512             # matmul free dim (one PSUM bank)
CFD = 2048           # cast-group free dim (4 banks)
R_TILES = NH // P    # 32 row tiles per core
N_G = MPTS // CFD    # 4 cast groups per row tile
N_C = MPTS // FD     # 16 col chunks
NEG_INIT = -3.0e38

_CACHE = {}


def _split_multi_waits(bir_json):
    """This container's walrus caps sync waits at 1 per instruction. Split any
    instruction carrying N>1 waits into N-1 single-wait NoOps (same engine,
    inserted just before it) plus the original with one wait."""
    import json

    d = json.loads(bir_json)
    count = 0
    for fn in d["functions"]:
        for blk in fn["blocks"]:
            out = []
            for ins in blk["instructions"]:
                si = ins.get("sync_info")
                waits = (si or {}).get("on_wait") or []
                if len(waits) > 1:
                    for w in waits[:-1]:
                        count += 1
                        out.append({
                            "debug": ins.get("debug", 0),
                            "engine": ins["engine"],
                            "ins": [],
                            "outs": [],
                            "name": f"waitsplit-{count}",
                            "opcode": "NoOp",
                            "sync_info": {"on_update": [], "on_wait": [w]},
                        })
                    si["on_wait"] = [waits[-1]]
                out.append(ins)
            blk["instructions"] = out
    return json.dumps(d).encode()


def _patch_compiler():
    """Route bass2jax's walrus invocation through _split_multi_waits."""
    import concourse.bass2jax as b2j

    if getattr(b2j, "_waitsplit_patched", False):
        return
    orig = b2j.compile_bir_kernel

    def patched(bir_json, *args, **kwargs):
        return orig(_split_multi_waits(bir_json), *args, **kwargs)

    b2j.compile_bir_kernel = patched
    b2j._waitsplit_patched = True


def _build_program():
    import concourse.bass as bass
    import concourse.tile as tile
    from concourse import mybir
    from contextlib import ExitStack

    _patch_compiler()

    f32 = mybir.dt.float32
    f32r = mybir.dt.float32r
    bf16 = mybir.dt.bfloat16

    nc = bass.Bass("TRN2", target_bir_lowering=False, debug=False)

    predT_d = nc.dram_tensor("predT", [5, NH], f32r, kind="ExternalInput").ap()
    targT_d = nc.dram_tensor("targT", [5, MPTS], f32r, kind="ExternalInput").ap()
    ident_d = nc.dram_tensor("ident", [P, P], bf16, kind="ExternalInput").ap()
    chamx_d = nc.dram_tensor("chamx", [P, R_TILES], f32, kind="ExternalOutput").ap()
    chamy_d = nc.dram_tensor("chamy", [P, N_C * 4], f32, kind="ExternalOutput").ap()

    with tile.TileContext(nc) as tc, ExitStack() as ctx:
        const_pool = ctx.enter_context(tc.tile_pool(name="const", bufs=1))
        acc_pool = ctx.enter_context(tc.tile_pool(name="acc", bufs=1))
        cast_pool = ctx.enter_context(tc.tile_pool(name="cast", bufs=4))
        scr_pool = ctx.enter_context(tc.tile_pool(name="scr", bufs=5))

        predT_sb = const_pool.tile([5, NH], f32r)
        targT_sb = const_pool.tile([5, MPTS], f32r)
        ident_sb = const_pool.tile([P, P], bf16)
        nc.sync.dma_start(predT_sb[:], predT_d[:])
        nc.sync.dma_start(targT_sb[:], targT_d[:])
        nc.sync.dma_start(ident_sb[:], ident_d[:])

        colacc = acc_pool.tile([P, MPTS], bf16)
        chamx_sb = acc_pool.tile([P, R_TILES], f32)
        chamy_sb = acc_pool.tile([P, N_C * 4], f32)

        with tc.tile_pool(name="mmpsum", bufs=2, space="PSUM") as mmp:
            for r in range(R_TILES):
                lhs = predT_sb[:, r * P:(r + 1) * P]
                rowcur = None
                for g in range(N_G):
                    pt = mmp.tile([P, CFD], f32, tag="mm")
                    for j in range(CFD // FD):
                        off = g * CFD + j * FD
                        nc.tensor.matmul(
                            pt[:, j * FD:(j + 1) * FD],
                            lhsT=lhs,
                            rhs=targT_sb[:, off:off + FD],
                            start=True,
                            stop=True,
                        )
                    # evacuate PSUM with a dtype cast on ACT
                    if r == 0:
                        cast_dst = colacc[:, g * CFD:(g + 1) * CFD]
                    elif g == 0:
                        cast_dst = scr_pool.tile([P, CFD], bf16, tag="row")
                    else:
                        cast_dst = cast_pool.tile([P, CFD], bf16, tag="cast")
                    nc.scalar.copy(cast_dst, pt[:])
                    # row path: running free-dim max (ping-pong chain)
                    if g == 0:
                        if r == 0:
                            rowcur = scr_pool.tile([P, CFD], bf16, tag="row")
                            nc.vector.tensor_copy(rowcur[:], cast_dst)
                        else:
                            rowcur = cast_dst
                    else:
                        nxt = scr_pool.tile([P, CFD], bf16, tag="row")
                        nc.vector.tensor_tensor(
                            out=nxt[:], in0=rowcur[:], in1=cast_dst,
                            op=mybir.AluOpType.max,
                        )
                        rowcur = nxt
                    # col path: elementwise max accumulate over row tiles
                    if r > 0:
                        cslice = colacc[:, g * CFD:(g + 1) * CFD]
                        nc.vector.tensor_tensor(
                            out=cslice, in0=cslice, in1=cast_dst,
                            op=mybir.AluOpType.max,
                        )
                nc.vector.tensor_reduce(
                    chamx_sb[:, r:r + 1], rowcur[:],
                    axis=mybir.AxisListType.X, op=mybir.AluOpType.max,
                )

        # partition-axis fold of colacc via PE transposes + free-dim reduces
        with tc.tile_pool(name="trpsum", bufs=4, space="PSUM") as trp:
            for c in range(N_C):
                tp = trp.tile([P, FD], bf16, tag="tr")
                for j in range(4):
                    src = colacc[:, c * FD + j * P: c * FD + (j + 1) * P]
                    nc.tensor.transpose(tp[:, j * P:(j + 1) * P], src, ident_sb[:])
                rr = tp[:].rearrange("p (j n) -> p j n", j=4)
                nc.vector.tensor_reduce(
                    chamy_sb[:, c * 4:(c + 1) * 4], rr,
                    axis=mybir.AxisListType.X, op=mybir.AluOpType.max,
                )

        nc.sync.dma_start(chamx_d[:], chamx_sb[:])
        nc.sync.dma_start(chamy_d[:], chamy_sb[:])

    return nc


def _augment(pred_b, target_b):
    """pred_b/target_b: [npts, 3] fp32 -> lhsT [5, n], rhs [5, m] fp32."""
    n = pred_b.shape[0]
    m = target_b.shape[0]
    lhs = np.empty((5, n), dtype=np.float32)
    lhs[0:3] = 2.0 * pred_b.T
    lhs[3] = np.sum(pred_b * pred_b, axis=1)
    lhs[4] = 1.0
    rhs = np.empty((5, m), dtype=np.float32)
    rhs[0:3] = target_b.T
    rhs[3] = -1.0
    rhs[4] = -np.sum(target_b * target_b, axis=1)
    return lhs, rhs


def kernel(pred, target):
    from concourse.bass_utils import run_bass_kernel_spmd

    pred = np.asarray(pred, dtype=np.float32)
    target = np.asarray(target, dtype=np.float32)
    assert pred.shape == (B, NPTS, 3) and target.shape == (B, MPTS, 3)

    if "nc" not in _CACHE:
        _CACHE["nc"] = _build_program()
    nc = _CACHE["nc"]

    ident = np.eye(P, dtype=ml_dtypes.bfloat16)
    in_maps = []
    for core in range(8):
        b, h = core // 2, core % 2
        lhs, rhs = _augment(pred[b, h * NH:(h + 1) * NH], target[b])
        in_maps.append({"predT": lhs, "targT": rhs, "ident": ident})

    res = run_bass_kernel_spmd(nc, in_maps, list(range(8)))

    cham_x = np.empty((B, NPTS), dtype=np.float32)
    chamy_part = np.empty((B, 2, MPTS), dtype=np.float32)
    for core in range(8):
        b, h = core // 2, core % 2
        out_x = res.results[core]["chamx"]          # [128, 32] nd2 max
        out_y = res.results[core]["chamy"]          # [128, 64] nd2 max partial
        # n = r*128 + p
        cham_x[b, h * NH:(h + 1) * NH] = np.maximum(-out_x.T.reshape(NH), 0.0)
        # m = c*512 + j*128 + p
        chamy_part[b, h] = np.transpose(
            out_y.reshape(P, N_C, 4), (1, 2, 0)
        ).reshape(MPTS)
    cham_y = np.maximum(-np.max(chamy_part, axis=1), 0.0)

    loss = cham_x.mean(axis=1).mean() + cham_y.mean(axis=1).mean()
    return np.asarray(loss, dtype=np.float32)


# revision 11
# speedup vs baseline: 1.2496x; 1.2496x over previous
"""Chamfer loss on 8 trn2 NeuronCores (Bass/Tile).

Reference computation (per batch b):
    d2[n, m] = ||pred[b,n] - target[b,m]||^2   (floored at 0)
    loss = mean_n min_m d2 + mean_m min_n d2,  averaged over batches.

Strategy (matches the data-parallel + N-tiling sharding hint):
  - 8 cores = 4 batches x 2 halves of N (rows of pred).
  - Core (b, h) computes nd2 = -d2 for its [4096 x 8192] block via a single
    K=5 augmented matmul on the PE:
        nd2[n, m] = 2 p.t - |p|^2 - |t|^2
    with lhsT = (2px, 2py, 2pz, |p|^2, 1), rhs = (tx, ty, tz, -1, -|t|^2),
    both fed as float32r (full-speed PE at free dim 512).
  - Row path (cham_x): running free-dim max of nd2 via fused
    tensor_tensor_reduce on DVE (bf16, after an ACT cast of PSUM->SBUF).
    Complete per core: no cross-core reduction.
  - Col path (cham_y partial): elementwise max accumulator [128, 8192] bf16
    over the 32 row tiles on DVE; final partition-axis fold via PE transposes
    + DVE reduces. Host min-reduces the two halves of each batch.
  - Host: tiny [128,32]+[128,64] outputs per core -> final scalar in numpy.
"""

import numpy as np
import ml_dtypes

B = 4
NPTS = 8192          # pred points per batch
MPTS = 8192          # target points per batch
NH = NPTS // 2       # rows per core
P = 128              # partitions
FD = 512             # matmul free dim (one PSUM bank)
CFD = 2048           # cast-group free dim (4 banks)
R_TILES = NH // P    # 32 row tiles per core
N_G = MPTS // CFD    # 4 cast groups per row tile
N_C = MPTS // FD     # 16 col chunks
K_AUG = 16           # augmented contraction dim (hi/lo compensated bf16)
GPSIMD_COL_GS = ()   # cast groups whose col-path max runs on GPSIMD
                     # (this walrus rejects TensorTensor on Pool: NCC_IXCG966)

_CACHE = {}


def _split_multi_waits(bir_json):
    """This container's walrus caps sync waits at 1 per instruction. Split any
    instruction carrying N>1 waits into N-1 single-wait NoOps (same engine,
    inserted just before it) plus the original with one wait."""
    import json

    d = json.loads(bir_json)
    count = 0
    for fn in d["functions"]:
        for blk in fn["blocks"]:
            out = []
            for ins in blk["instructions"]:
                si = ins.get("sync_info")
                waits = (si or {}).get("on_wait") or []
                if len(waits) > 1:
                    for w in waits[:-1]:
                        count += 1
                        out.append({
                            "debug": ins.get("debug", 0),
                            "engine": ins["engine"],
                            "ins": [],
                            "outs": [],
                            "name": f"waitsplit-{count}",
                            "opcode": "NoOp",
                            "sync_info": {"on_update": [], "on_wait": [w]},
                        })
                    si["on_wait"] = [waits[-1]]
                out.append(ins)
            blk["instructions"] = out
    return json.dumps(d).encode()


def _patch_compiler():
    """Route bass2jax's walrus invocation through _split_multi_waits."""
    import concourse.bass2jax as b2j

    if getattr(b2j, "_waitsplit_patched", False):
        return
    orig = b2j.compile_bir_kernel

    def patched(bir_json, *args, **kwargs):
        return orig(_split_multi_waits(bir_json), *args, **kwargs)

    b2j.compile_bir_kernel = patched
    b2j._waitsplit_patched = True


def _build_program():
    import concourse.bass as bass
    import concourse.tile as tile
    from concourse import mybir
    from contextlib import ExitStack

    _patch_compiler()

    f32 = mybir.dt.float32
    bf16 = mybir.dt.bfloat16

    nc = bass.Bass("TRN2", target_bir_lowering=False, debug=False)

    predT_d = nc.dram_tensor("predT", [K_AUG, NH], bf16, kind="ExternalInput").ap()
    targT_d = nc.dram_tensor("targT", [K_AUG, MPTS], bf16, kind="ExternalInput").ap()
    ident_d = nc.dram_tensor("ident", [P, P], bf16, kind="ExternalInput").ap()
    chamx_d = nc.dram_tensor("chamx", [P, R_TILES], f32, kind="ExternalOutput").ap()
    chamy_d = nc.dram_tensor("chamy", [P, N_C * 4], f32, kind="ExternalOutput").ap()

    with tile.TileContext(nc) as tc, ExitStack() as ctx:
        const_pool = ctx.enter_context(tc.tile_pool(name="const", bufs=1))
        acc_pool = ctx.enter_context(tc.tile_pool(name="acc", bufs=1))
        cast_pool = ctx.enter_context(tc.tile_pool(name="cast", bufs=4))
        scr_pool = ctx.enter_context(tc.tile_pool(name="scr", bufs=5))

        predT_sb = const_pool.tile([K_AUG, NH], bf16)
        targT_sb = const_pool.tile([K_AUG, MPTS], bf16)
        ident_sb = const_pool.tile([P, P], bf16)
        nc.sync.dma_start(predT_sb[:], predT_d[:])
        nc.sync.dma_start(targT_sb[:], targT_d[:])
        nc.sync.dma_start(ident_sb[:], ident_d[:])

        colacc = acc_pool.tile([P, MPTS], bf16)
        chamx_sb = acc_pool.tile([P, R_TILES], f32)
        chamy_sb = acc_pool.tile([P, N_C * 4], f32)

        with tc.tile_pool(name="mmpsum", bufs=2, space="PSUM") as mmp:
            for r in range(R_TILES):
                lhs = predT_sb[:, r * P:(r + 1) * P]
                rowcur = None
                for g in range(N_G):
                    pt = mmp.tile([P, CFD], f32, tag="mm")
                    for j in range(CFD // FD):
                        off = g * CFD + j * FD
                        nc.tensor.matmul(
                            pt[:, j * FD:(j + 1) * FD],
                            lhsT=lhs,
                            rhs=targT_sb[:, off:off + FD],
                            start=True,
                            stop=True,
                        )
                    # evacuate PSUM with a dtype cast on ACT
                    if r == 0:
                        cast_dst = colacc[:, g * CFD:(g + 1) * CFD]
                    elif g == 0:
                        cast_dst = scr_pool.tile([P, CFD], bf16, tag="row")
                    else:
                        cast_dst = cast_pool.tile([P, CFD], bf16, tag="cast")
                    nc.scalar.copy(cast_dst, pt[:])
                    # row path: running free-dim max (ping-pong chain)
                    if g == 0:
                        if r == 0:
                            rowcur = scr_pool.tile([P, CFD], bf16, tag="row")
                            nc.vector.tensor_copy(rowcur[:], cast_dst)
                        else:
                            rowcur = cast_dst
                    else:
                        nxt = scr_pool.tile([P, CFD], bf16, tag="row")
                        nc.vector.tensor_tensor(
                            out=nxt[:], in0=rowcur[:], in1=cast_dst,
                            op=mybir.AluOpType.max,
                        )
                        rowcur = nxt
                    # col path: elementwise max accumulate over row tiles;
                    # part of it runs on otherwise-idle GPSIMD
                    if r > 0:
                        cslice = colacc[:, g * CFD:(g + 1) * CFD]
                        eng = nc.gpsimd if g in GPSIMD_COL_GS else nc.vector
                        eng.tensor_tensor(
                            out=cslice, in0=cslice, in1=cast_dst,
                            op=mybir.AluOpType.max,
                        )
                # fold rowcur [P, CFD] -> [P, FD] pairwise, then one reduce
                f1 = scr_pool.tile([P, CFD // 2], bf16, tag="fold1")
                nc.vector.tensor_tensor(
                    out=f1[:], in0=rowcur[:, :CFD // 2], in1=rowcur[:, CFD // 2:],
                    op=mybir.AluOpType.max,
                )
                f2 = scr_pool.tile([P, CFD // 4], bf16, tag="fold2")
                nc.vector.tensor_tensor(
                    out=f2[:], in0=f1[:, :CFD // 4], in1=f1[:, CFD // 4:],
                    op=mybir.AluOpType.max,
                )
                nc.vector.tensor_reduce(
                    chamx_sb[:, r:r + 1], f2[:],
                    axis=mybir.AxisListType.X, op=mybir.AluOpType.max,
                )

        # partition-axis fold of colacc via PE transposes + free-dim reduces
        with tc.tile_pool(name="trpsum", bufs=4, space="PSUM") as trp:
            for c in range(N_C):
                tp = trp.tile([P, FD], bf16, tag="tr")
                for j in range(4):
                    src = colacc[:, c * FD + j * P: c * FD + (j + 1) * P]
                    nc.tensor.transpose(tp[:, j * P:(j + 1) * P], src, ident_sb[:])
                rr = tp[:].rearrange("p (j n) -> p j n", j=4)
                nc.vector.tensor_reduce(
                    chamy_sb[:, c * 4:(c + 1) * 4], rr,
                    axis=mybir.AxisListType.X, op=mybir.AluOpType.max,
                )

        nc.sync.dma_start(chamx_d[:], chamx_sb[:])
        nc.sync.dma_start(chamy_d[:], chamy_sb[:])

    return nc


def _augment(pred_b, target_b):
    """Hi/lo-compensated bf16 augmentation so a K=16 bf16 matmul reproduces
    nd2 = 2 p.t - |p|^2 - |t|^2 to ~1e-5 absolute despite bf16 inputs.

    pred_b/target_b: [npts, 3] fp32 -> lhsT [16, n], rhs [16, m] bf16."""
    bft = ml_dtypes.bfloat16

    def hilo(x):
        h = x.astype(bft).astype(np.float32)
        l = (x - h).astype(bft).astype(np.float32)
        return h, l

    p = np.asarray(pred_b, dtype=np.float32)
    t = np.asarray(target_b, dtype=np.float32)
    ph, pl = hilo(p)
    th, tl = hilo(t)
    p2h, p2l = hilo(np.sum(p * p, axis=1))
    t2h, t2l = hilo(np.sum(t * t, axis=1))
    n, m = p.shape[0], t.shape[0]
    L = np.zeros((K_AUG, n), np.float32)
    R = np.zeros((K_AUG, m), np.float32)
    L[0:3] = 2.0 * ph.T
    R[0:3] = th.T
    L[3:6] = 2.0 * ph.T
    R[3:6] = tl.T
    L[6:9] = 2.0 * pl.T
    R[6:9] = th.T
    L[9:12] = 2.0 * pl.T
    R[9:12] = tl.T
    L[12] = p2h
    R[12] = -1.0
    L[13] = p2l
    R[13] = -1.0
    L[14] = 1.0
    R[14] = -t2h
    L[15] = 1.0
    R[15] = -t2l
    return L.astype(bft), R.astype(bft)


def kernel(pred, target):
    from concourse.bass_utils import run_bass_kernel_spmd

    pred = np.asarray(pred, dtype=np.float32)
    target = np.asarray(target, dtype=np.float32)
    assert pred.shape == (B, NPTS, 3) and target.shape == (B, MPTS, 3)

    if "nc" not in _CACHE:
        _CACHE["nc"] = _build_program()
    nc = _CACHE["nc"]

    ident = np.eye(P, dtype=ml_dtypes.bfloat16)
    in_maps = []
    for core in range(8):
        b, h = core // 2, core % 2
        lhs, rhs = _augment(pred[b, h * NH:(h + 1) * NH], target[b])
        in_maps.append({"predT": lhs, "targT": rhs, "ident": ident})

    res = run_bass_kernel_spmd(nc, in_maps, list(range(8)))

    cham_x = np.empty((B, NPTS), dtype=np.float32)
    chamy_part = np.empty((B, 2, MPTS), dtype=np.float32)
    for core in range(8):
        b, h = core // 2, core % 2
        out_x = res.results[core]["chamx"]          # [128, 32] nd2 max
        out_y = res.results[core]["chamy"]          # [128, 64] nd2 max partial
        # n = r*128 + p
        cham_x[b, h * NH:(h + 1) * NH] = np.maximum(-out_x.T.reshape(NH), 0.0)
        # m = c*512 + j*128 + p
        chamy_part[b, h] = np.transpose(
            out_y.reshape(P, N_C, 4), (1, 2, 0)
        ).reshape(MPTS)
    cham_y = np.maximum(-np.max(chamy_part, axis=1), 0.0)

    loss = cham_x.mean(axis=1).mean() + cham_y.mean(axis=1).mean()
    return np.asarray(loss, dtype=np.float32)


# revision 14
# speedup vs baseline: 1.2569x; 1.0058x over previous
"""Chamfer loss on 8 trn2 NeuronCores (Bass/Tile).

Reference computation (per batch b):
    d2[n, m] = ||pred[b,n] - target[b,m]||^2   (floored at 0)
    loss = mean_n min_m d2 + mean_m min_n d2,  averaged over batches.

Strategy (matches the data-parallel + N-tiling sharding hint):
  - 8 cores = 4 batches x 2 halves of N (rows of pred).
  - Core (b, h) computes nd2 = -d2 for its [4096 x 8192] block via a single
    K=5 augmented matmul on the PE:
        nd2[n, m] = 2 p.t - |p|^2 - |t|^2
    with lhsT = (2px, 2py, 2pz, |p|^2, 1), rhs = (tx, ty, tz, -1, -|t|^2),
    both fed as float32r (full-speed PE at free dim 512).
  - Row path (cham_x): running free-dim max of nd2 via fused
    tensor_tensor_reduce on DVE (bf16, after an ACT cast of PSUM->SBUF).
    Complete per core: no cross-core reduction.
  - Col path (cham_y partial): elementwise max accumulator [128, 8192] bf16
    over the 32 row tiles on DVE; final partition-axis fold via PE transposes
    + DVE reduces. Host min-reduces the two halves of each batch.
  - Host: tiny [128,32]+[128,64] outputs per core -> final scalar in numpy.
"""

import numpy as np
import ml_dtypes

B = 4
NPTS = 8192          # pred points per batch
MPTS = 8192          # target points per batch
NH = NPTS // 2       # rows per core
P = 128              # partitions
FD = 512             # matmul free dim (one PSUM bank)
CFD = 2048           # cast-group free dim (4 banks)
R_TILES = NH // P    # 32 row tiles per core
N_G = MPTS // CFD    # 4 cast groups per row tile
N_C = MPTS // FD     # 16 col chunks
K_AUG = 16           # augmented contraction dim (hi/lo compensated bf16)
GPSIMD_COL_GS = ()   # cast groups whose col-path max runs on GPSIMD
                     # (this walrus rejects TensorTensor on Pool: NCC_IXCG966)

_CACHE = {}


def _split_multi_waits(bir_json):
    """This container's walrus caps sync waits at 1 per instruction. Split any
    instruction carrying N>1 waits into N-1 single-wait NoOps (same engine,
    inserted just before it) plus the original with one wait."""
    import json

    d = json.loads(bir_json)
    count = 0
    for fn in d["functions"]:
        for blk in fn["blocks"]:
            out = []
            for ins in blk["instructions"]:
                si = ins.get("sync_info")
                waits = (si or {}).get("on_wait") or []
                if len(waits) > 1:
                    for w in waits[:-1]:
                        count += 1
                        out.append({
                            "debug": ins.get("debug", 0),
                            "engine": ins["engine"],
                            "ins": [],
                            "outs": [],
                            "name": f"waitsplit-{count}",
                            "opcode": "NoOp",
                            "sync_info": {"on_update": [], "on_wait": [w]},
                        })
                    si["on_wait"] = [waits[-1]]
                out.append(ins)
            blk["instructions"] = out
    return json.dumps(d).encode()


def _patch_compiler():
    """Route bass2jax's walrus invocation through _split_multi_waits."""
    import concourse.bass2jax as b2j

    if getattr(b2j, "_waitsplit_patched", False):
        return
    orig = b2j.compile_bir_kernel

    def patched(bir_json, *args, **kwargs):
        return orig(_split_multi_waits(bir_json), *args, **kwargs)

    b2j.compile_bir_kernel = patched
    b2j._waitsplit_patched = True


def _build_program():
    import concourse.bass as bass
    import concourse.tile as tile
    from concourse import mybir
    from contextlib import ExitStack

    _patch_compiler()

    f32 = mybir.dt.float32
    bf16 = mybir.dt.bfloat16

    nc = bass.Bass("TRN2", target_bir_lowering=False, debug=False)

    predT_d = nc.dram_tensor("predT", [K_AUG, NH], bf16, kind="ExternalInput").ap()
    targT_d = nc.dram_tensor("targT", [K_AUG, MPTS], bf16, kind="ExternalInput").ap()
    ident_d = nc.dram_tensor("ident", [P, P], bf16, kind="ExternalInput").ap()
    chamx_d = nc.dram_tensor("chamx", [P, R_TILES], f32, kind="ExternalOutput").ap()
    chamy_d = nc.dram_tensor("chamy", [P, N_C * 4], f32, kind="ExternalOutput").ap()

    with tile.TileContext(nc) as tc, ExitStack() as ctx:
        const_pool = ctx.enter_context(tc.tile_pool(name="const", bufs=1))
        acc_pool = ctx.enter_context(tc.tile_pool(name="acc", bufs=1))
        cast_pool = ctx.enter_context(tc.tile_pool(name="cast", bufs=6))
        scr_pool = ctx.enter_context(tc.tile_pool(name="scr", bufs=5))

        predT_sb = const_pool.tile([K_AUG, NH], bf16)
        targT_sb = const_pool.tile([K_AUG, MPTS], bf16)
        ident_sb = const_pool.tile([P, P], bf16)
        # chunked loads so the first matmuls start as soon as their slice lands
        nc.sync.dma_start(predT_sb[:, :P], predT_d[:, :P])
        for g in range(N_G):
            sl = slice(g * CFD, (g + 1) * CFD)
            nc.sync.dma_start(targT_sb[:, sl], targT_d[:, sl])
        nc.sync.dma_start(predT_sb[:, P:], predT_d[:, P:])
        nc.sync.dma_start(ident_sb[:], ident_d[:])

        colacc = acc_pool.tile([P, MPTS], bf16)
        chamx_sb = acc_pool.tile([P, R_TILES], f32)
        chamy_sb = acc_pool.tile([P, N_C * 4], f32)

        with tc.tile_pool(name="mmpsum", bufs=2, space="PSUM") as mmp:
            for r in range(R_TILES):
                lhs = predT_sb[:, r * P:(r + 1) * P]
                rowcur = None
                for g in range(N_G):
                    pt = mmp.tile([P, CFD], f32, tag="mm")
                    for j in range(CFD // FD):
                        off = g * CFD + j * FD
                        nc.tensor.matmul(
                            pt[:, j * FD:(j + 1) * FD],
                            lhsT=lhs,
                            rhs=targT_sb[:, off:off + FD],
                            start=True,
                            stop=True,
                        )
                    # evacuate PSUM with a dtype cast on ACT
                    if r == 0:
                        cast_dst = colacc[:, g * CFD:(g + 1) * CFD]
                    elif g == 0:
                        cast_dst = scr_pool.tile([P, CFD], bf16, tag="row")
                    else:
                        cast_dst = cast_pool.tile([P, CFD], bf16, tag="cast")
                    nc.scalar.copy(cast_dst, pt[:])
                    # row path: running free-dim max (ping-pong chain)
                    if g == 0:
                        if r == 0:
                            rowcur = scr_pool.tile([P, CFD], bf16, tag="row")
                            nc.vector.tensor_copy(rowcur[:], cast_dst)
                        else:
                            rowcur = cast_dst
                    else:
                        nxt = scr_pool.tile([P, CFD], bf16, tag="row")
                        nc.vector.tensor_tensor(
                            out=nxt[:], in0=rowcur[:], in1=cast_dst,
                            op=mybir.AluOpType.max,
                        )
                        rowcur = nxt
                    # col path: elementwise max accumulate over row tiles;
                    # part of it runs on otherwise-idle GPSIMD
                    if r > 0:
                        cslice = colacc[:, g * CFD:(g + 1) * CFD]
                        eng = nc.gpsimd if g in GPSIMD_COL_GS else nc.vector
                        eng.tensor_tensor(
                            out=cslice, in0=cslice, in1=cast_dst,
                            op=mybir.AluOpType.max,
                        )
                # fold rowcur [P, CFD] -> [P, FD] pairwise, then one reduce
                f1 = scr_pool.tile([P, CFD // 2], bf16, tag="fold1")
                nc.vector.tensor_tensor(
                    out=f1[:], in0=rowcur[:, :CFD // 2], in1=rowcur[:, CFD // 2:],
                    op=mybir.AluOpType.max,
                )
                # keep folding down to 64 before the (1x-rate) reduce
                prev = f1
                w = CFD // 2
                while w > 64:
                    nxt = scr_pool.tile([P, w // 2], bf16, tag=f"fold{w // 2}")
                    nc.vector.tensor_tensor(
                        out=nxt[:], in0=prev[:, :w // 2], in1=prev[:, w // 2:],
                        op=mybir.AluOpType.max,
                    )
                    prev = nxt
                    w //= 2
                nc.vector.tensor_reduce(
                    chamx_sb[:, r:r + 1], prev[:],
                    axis=mybir.AxisListType.X, op=mybir.AluOpType.max,
                )

        # partition-axis fold of colacc via PE transposes + free-dim reduces
        with tc.tile_pool(name="trpsum", bufs=4, space="PSUM") as trp:
            for c in range(N_C):
                tp = trp.tile([P, FD], bf16, tag="tr")
                for j in range(4):
                    src = colacc[:, c * FD + j * P: c * FD + (j + 1) * P]
                    nc.tensor.transpose(tp[:, j * P:(j + 1) * P], src, ident_sb[:])
                rr = tp[:].rearrange("p (j n) -> p j n", j=4)
                nc.vector.tensor_reduce(
                    chamy_sb[:, c * 4:(c + 1) * 4], rr,
                    axis=mybir.AxisListType.X, op=mybir.AluOpType.max,
                )

        nc.sync.dma_start(chamx_d[:], chamx_sb[:])
        nc.sync.dma_start(chamy_d[:], chamy_sb[:])

    return nc


def _augment(pred_b, target_b):
    """Hi/lo-compensated bf16 augmentation so a K=16 bf16 matmul reproduces
    nd2 = 2 p.t - |p|^2 - |t|^2 to ~1e-5 absolute despite bf16 inputs.

    pred_b/target_b: [npts, 3] fp32 -> lhsT [16, n], rhs [16, m] bf16."""
    bft = ml_dtypes.bfloat16

    def hilo(x):
        h = x.astype(bft).astype(np.float32)
        l = (x - h).astype(bft).astype(np.float32)
        return h, l

    p = np.asarray(pred_b, dtype=np.float32)
    t = np.asarray(target_b, dtype=np.float32)
    ph, pl = hilo(p)
    th, tl = hilo(t)
    p2h, p2l = hilo(np.sum(p * p, axis=1))
    t2h, t2l = hilo(np.sum(t * t, axis=1))
    n, m = p.shape[0], t.shape[0]
    L = np.zeros((K_AUG, n), np.float32)
    R = np.zeros((K_AUG, m), np.float32)
    L[0:3] = 2.0 * ph.T
    R[0:3] = th.T
    L[3:6] = 2.0 * ph.T
    R[3:6] = tl.T
    L[6:9] = 2.0 * pl.T
    R[6:9] = th.T
    L[9:12] = 2.0 * pl.T
    R[9:12] = tl.T
    L[12] = p2h
    R[12] = -1.0
    L[13] = p2l
    R[13] = -1.0
    L[14] = 1.0
    R[14] = -t2h
    L[15] = 1.0
    R[15] = -t2l
    return L.astype(bft), R.astype(bft)


def kernel(pred, target):
    from concourse.bass_utils import run_bass_kernel_spmd

    pred = np.asarray(pred, dtype=np.float32)
    target = np.asarray(target, dtype=np.float32)
    assert pred.shape == (B, NPTS, 3) and target.shape == (B, MPTS, 3)

    if "nc" not in _CACHE:
        _CACHE["nc"] = _build_program()
    nc = _CACHE["nc"]

    ident = np.eye(P, dtype=ml_dtypes.bfloat16)
    in_maps = []
    for core in range(8):
        b, h = core // 2, core % 2
        lhs, rhs = _augment(pred[b, h * NH:(h + 1) * NH], target[b])
        in_maps.append({"predT": lhs, "targT": rhs, "ident": ident})

    res = run_bass_kernel_spmd(nc, in_maps, list(range(8)))

    cham_x = np.empty((B, NPTS), dtype=np.float32)
    chamy_part = np.empty((B, 2, MPTS), dtype=np.float32)
    for core in range(8):
        b, h = core // 2, core % 2
        out_x = res.results[core]["chamx"]          # [128, 32] nd2 max
        out_y = res.results[core]["chamy"]          # [128, 64] nd2 max partial
        # n = r*128 + p
        cham_x[b, h * NH:(h + 1) * NH] = np.maximum(-out_x.T.reshape(NH), 0.0)
        # m = c*512 + j*128 + p
        chamy_part[b, h] = np.transpose(
            out_y.reshape(P, N_C, 4), (1, 2, 0)
        ).reshape(MPTS)
    cham_y = np.maximum(-np.max(chamy_part, axis=1), 0.0)

    loss = cham_x.mean(axis=1).mean() + cham_y.mean(axis=1).mean()
    return np.asarray(loss, dtype=np.float32)


# revision 15
# speedup vs baseline: 1.2573x; 1.0004x over previous
"""Chamfer loss on 8 trn2 NeuronCores (Bass/Tile).

Reference computation (per batch b):
    d2[n, m] = ||pred[b,n] - target[b,m]||^2   (floored at 0)
    loss = mean_n min_m d2 + mean_m min_n d2,  averaged over batches.

Strategy (matches the data-parallel + N-tiling sharding hint):
  - 8 cores = 4 batches x 2 halves of N (rows of pred).
  - Core (b, h) computes nd2 = -d2 for its [4096 x 8192] block via a single
    K=5 augmented matmul on the PE:
        nd2[n, m] = 2 p.t - |p|^2 - |t|^2
    with lhsT = (2px, 2py, 2pz, |p|^2, 1), rhs = (tx, ty, tz, -1, -|t|^2),
    both fed as float32r (full-speed PE at free dim 512).
  - Row path (cham_x): running free-dim max of nd2 via fused
    tensor_tensor_reduce on DVE (bf16, after an ACT cast of PSUM->SBUF).
    Complete per core: no cross-core reduction.
  - Col path (cham_y partial): elementwise max accumulator [128, 8192] bf16
    over the 32 row tiles on DVE; final partition-axis fold via PE transposes
    + DVE reduces. Host min-reduces the two halves of each batch.
  - Host: tiny [128,32]+[128,64] outputs per core -> final scalar in numpy.
"""

import numpy as np
import ml_dtypes

B = 4
NPTS = 8192          # pred points per batch
MPTS = 8192          # target points per batch
NH = NPTS // 2       # rows per core
P = 128              # partitions
FD = 512             # matmul free dim (one PSUM bank)
CFD = 2048           # cast-group free dim (4 banks)
R_TILES = NH // P    # 32 row tiles per core
N_G = MPTS // CFD    # 4 cast groups per row tile
N_C = MPTS // FD     # 16 col chunks
K_AUG = 16           # augmented contraction dim (hi/lo compensated bf16)
GPSIMD_COL_GS = ()   # cast groups whose col-path max runs on GPSIMD
                     # (this walrus rejects TensorTensor on Pool: NCC_IXCG966)

_CACHE = {}


def _split_multi_waits(bir_json):
    """This container's walrus caps sync waits at 1 per instruction. Split any
    instruction carrying N>1 waits into N-1 single-wait NoOps (same engine,
    inserted just before it) plus the original with one wait."""
    import json

    d = json.loads(bir_json)
    count = 0
    for fn in d["functions"]:
        for blk in fn["blocks"]:
            out = []
            for ins in blk["instructions"]:
                si = ins.get("sync_info")
                waits = (si or {}).get("on_wait") or []
                if len(waits) > 1:
                    for w in waits[:-1]:
                        count += 1
                        out.append({
                            "debug": ins.get("debug", 0),
                            "engine": ins["engine"],
                            "ins": [],
                            "outs": [],
                            "name": f"waitsplit-{count}",
                            "opcode": "NoOp",
                            "sync_info": {"on_update": [], "on_wait": [w]},
                        })
                    si["on_wait"] = [waits[-1]]
                out.append(ins)
            blk["instructions"] = out
    return json.dumps(d).encode()


def _patch_compiler():
    """Route bass2jax's walrus invocation through _split_multi_waits."""
    import concourse.bass2jax as b2j

    if getattr(b2j, "_waitsplit_patched", False):
        return
    orig = b2j.compile_bir_kernel

    def patched(bir_json, *args, **kwargs):
        return orig(_split_multi_waits(bir_json), *args, **kwargs)

    b2j.compile_bir_kernel = patched
    b2j._waitsplit_patched = True


def _build_program():
    import concourse.bass as bass
    import concourse.tile as tile
    from concourse import mybir
    from contextlib import ExitStack

    _patch_compiler()

    f32 = mybir.dt.float32
    bf16 = mybir.dt.bfloat16

    nc = bass.Bass("TRN2", target_bir_lowering=False, debug=False)

    predT_d = nc.dram_tensor("predT", [K_AUG, NH], bf16, kind="ExternalInput").ap()
    targT_d = nc.dram_tensor("targT", [K_AUG, MPTS], bf16, kind="ExternalInput").ap()
    ident_d = nc.dram_tensor("ident", [P, P], bf16, kind="ExternalInput").ap()
    chamx_d = nc.dram_tensor("chamx", [P, R_TILES], f32, kind="ExternalOutput").ap()
    chamy_d = nc.dram_tensor("chamy", [P, N_C * 4], f32, kind="ExternalOutput").ap()

    with tile.TileContext(nc) as tc, ExitStack() as ctx:
        const_pool = ctx.enter_context(tc.tile_pool(name="const", bufs=1))
        acc_pool = ctx.enter_context(tc.tile_pool(name="acc", bufs=1))
        cast_pool = ctx.enter_context(tc.tile_pool(name="cast", bufs=6))
        scr_pool = ctx.enter_context(tc.tile_pool(name="scr", bufs=5))

        predT_sb = const_pool.tile([K_AUG, NH], bf16)
        targT_sb = const_pool.tile([K_AUG, MPTS], bf16)
        ident_sb = const_pool.tile([P, P], bf16)
        # chunked loads so the first matmuls start as soon as their slice lands
        nc.sync.dma_start(predT_sb[:, :P], predT_d[:, :P])
        for g in range(N_G):
            sl = slice(g * CFD, (g + 1) * CFD)
            nc.sync.dma_start(targT_sb[:, sl], targT_d[:, sl])
        nc.sync.dma_start(predT_sb[:, P:], predT_d[:, P:])
        nc.sync.dma_start(ident_sb[:], ident_d[:])

        colacc = acc_pool.tile([P, MPTS], bf16)
        chamx_sb = acc_pool.tile([P, R_TILES], f32)
        chamy_sb = acc_pool.tile([P, N_C * 4], f32)

        with tc.tile_pool(name="mmpsum", bufs=2, space="PSUM") as mmp:
            for r in range(R_TILES):
                lhs = predT_sb[:, r * P:(r + 1) * P]
                rowcur = None
                for g in range(N_G):
                    pt = mmp.tile([P, CFD], f32, tag="mm")
                    for j in range(CFD // FD):
                        off = g * CFD + j * FD
                        nc.tensor.matmul(
                            pt[:, j * FD:(j + 1) * FD],
                            lhsT=lhs,
                            rhs=targT_sb[:, off:off + FD],
                            start=True,
                            stop=True,
                        )
                    # evacuate PSUM with a dtype cast on ACT
                    if r == 0:
                        cast_dst = colacc[:, g * CFD:(g + 1) * CFD]
                    elif g == 0:
                        cast_dst = scr_pool.tile([P, CFD], bf16, tag="row")
                    else:
                        cast_dst = cast_pool.tile([P, CFD], bf16, tag="cast")
                    nc.scalar.copy(cast_dst, pt[:])
                    # row path: running free-dim max (ping-pong chain)
                    if g == 0:
                        rowcur = cast_dst
                    else:
                        nxt = scr_pool.tile([P, CFD], bf16, tag="row")
                        nc.vector.tensor_tensor(
                            out=nxt[:], in0=rowcur[:], in1=cast_dst,
                            op=mybir.AluOpType.max,
                        )
                        rowcur = nxt
                    # col path: elementwise max accumulate over row tiles;
                    # part of it runs on otherwise-idle GPSIMD
                    if r > 0:
                        cslice = colacc[:, g * CFD:(g + 1) * CFD]
                        eng = nc.gpsimd if g in GPSIMD_COL_GS else nc.vector
                        eng.tensor_tensor(
                            out=cslice, in0=cslice, in1=cast_dst,
                            op=mybir.AluOpType.max,
                        )
                # fold rowcur [P, CFD] -> [P, FD] pairwise, then one reduce
                f1 = scr_pool.tile([P, CFD // 2], bf16, tag="fold1")
                nc.vector.tensor_tensor(
                    out=f1[:], in0=rowcur[:, :CFD // 2], in1=rowcur[:, CFD // 2:],
                    op=mybir.AluOpType.max,
                )
                # keep folding down to 64 before the (1x-rate) reduce
                prev = f1
                w = CFD // 2
                while w > 64:
                    nxt = scr_pool.tile([P, w // 2], bf16, tag=f"fold{w // 2}")
                    nc.vector.tensor_tensor(
                        out=nxt[:], in0=prev[:, :w // 2], in1=prev[:, w // 2:],
                        op=mybir.AluOpType.max,
                    )
                    prev = nxt
                    w //= 2
                nc.vector.tensor_reduce(
                    chamx_sb[:, r:r + 1], prev[:],
                    axis=mybir.AxisListType.X, op=mybir.AluOpType.max,
                )

        # partition-axis fold of colacc via PE transposes + free-dim reduces
        with tc.tile_pool(name="trpsum", bufs=4, space="PSUM") as trp:
            for c in range(N_C):
                tp = trp.tile([P, FD], bf16, tag="tr")
                for j in range(4):
                    src = colacc[:, c * FD + j * P: c * FD + (j + 1) * P]
                    nc.tensor.transpose(tp[:, j * P:(j + 1) * P], src, ident_sb[:])
                rr = tp[:].rearrange("p (j n) -> p j n", j=4)
                nc.vector.tensor_reduce(
                    chamy_sb[:, c * 4:(c + 1) * 4], rr,
                    axis=mybir.AxisListType.X, op=mybir.AluOpType.max,
                )

        nc.sync.dma_start(chamx_d[:], chamx_sb[:])
        nc.sync.dma_start(chamy_d[:], chamy_sb[:])

    return nc


def _augment(pred_b, target_b):
    """Hi/lo-compensated bf16 augmentation so a K=16 bf16 matmul reproduces
    nd2 = 2 p.t - |p|^2 - |t|^2 to ~1e-5 absolute despite bf16 inputs.

    pred_b/target_b: [npts, 3] fp32 -> lhsT [16, n], rhs [16, m] bf16."""
    bft = ml_dtypes.bfloat16

    def hilo(x):
        h = x.astype(bft).astype(np.float32)
        l = (x - h).astype(bft).astype(np.float32)
        return h, l

    p = np.asarray(pred_b, dtype=np.float32)
    t = np.asarray(target_b, dtype=np.float32)
    ph, pl = hilo(p)
    th, tl = hilo(t)
    p2h, p2l = hilo(np.sum(p * p, axis=1))
    t2h, t2l = hilo(np.sum(t * t, axis=1))
    n, m = p.shape[0], t.shape[0]
    L = np.zeros((K_AUG, n), np.float32)
    R = np.zeros((K_AUG, m), np.float32)
    L[0:3] = 2.0 * ph.T
    R[0:3] = th.T
    L[3:6] = 2.0 * ph.T
    R[3:6] = tl.T
    L[6:9] = 2.0 * pl.T
    R[6:9] = th.T
    L[9:12] = 2.0 * pl.T
    R[9:12] = tl.T
    L[12] = p2h
    R[12] = -1.0
    L[13] = p2l
    R[13] = -1.0
    L[14] = 1.0
    R[14] = -t2h
    L[15] = 1.0
    R[15] = -t2l
    return L.astype(bft), R.astype(bft)


def kernel(pred, target):
    from concourse.bass_utils import run_bass_kernel_spmd

    pred = np.asarray(pred, dtype=np.float32)
    target = np.asarray(target, dtype=np.float32)
    assert pred.shape == (B, NPTS, 3) and target.shape == (B, MPTS, 3)

    if "nc" not in _CACHE:
        _CACHE["nc"] = _build_program()
    nc = _CACHE["nc"]

    ident = np.eye(P, dtype=ml_dtypes.bfloat16)
    in_maps = []
    for core in range(8):
        b, h = core // 2, core % 2
        lhs, rhs = _augment(pred[b, h * NH:(h + 1) * NH], target[b])
        in_maps.append({"predT": lhs, "targT": rhs, "ident": ident})

    res = run_bass_kernel_spmd(nc, in_maps, list(range(8)))

    cham_x = np.empty((B, NPTS), dtype=np.float32)
    chamy_part = np.empty((B, 2, MPTS), dtype=np.float32)
    for core in range(8):
        b, h = core // 2, core % 2
        out_x = res.results[core]["chamx"]          # [128, 32] nd2 max
        out_y = res.results[core]["chamy"]          # [128, 64] nd2 max partial
        # n = r*128 + p
        cham_x[b, h * NH:(h + 1) * NH] = np.maximum(-out_x.T.reshape(NH), 0.0)
        # m = c*512 + j*128 + p
        chamy_part[b, h] = np.transpose(
            out_y.reshape(P, N_C, 4), (1, 2, 0)
        ).reshape(MPTS)
    cham_y = np.maximum(-np.max(chamy_part, axis=1), 0.0)

    loss = cham_x.mean(axis=1).mean() + cham_y.mean(axis=1).mean()
    return np.asarray(loss, dtype=np.float32)


# revision 17
# speedup vs baseline: 1.3136x; 1.0447x over previous
"""Chamfer loss on 8 trn2 NeuronCores (Bass/Tile).

Reference computation (per batch b):
    d2[n, m] = ||pred[b,n] - target[b,m]||^2   (floored at 0)
    loss = mean_n min_m d2 + mean_m min_n d2,  averaged over batches.

Strategy (matches the data-parallel + N-tiling sharding hint):
  - 8 cores = 4 batches x 2 halves of N (rows of pred).
  - Core (b, h) computes nd2 = -d2 for its [4096 x 8192] block via a single
    K=5 augmented matmul on the PE:
        nd2[n, m] = 2 p.t - |p|^2 - |t|^2
    with lhsT = (2px, 2py, 2pz, |p|^2, 1), rhs = (tx, ty, tz, -1, -|t|^2),
    both fed as float32r (full-speed PE at free dim 512).
  - Row path (cham_x): running free-dim max of nd2 via fused
    tensor_tensor_reduce on DVE (bf16, after an ACT cast of PSUM->SBUF).
    Complete per core: no cross-core reduction.
  - Col path (cham_y partial): elementwise max accumulator [128, 8192] bf16
    over the 32 row tiles on DVE; final partition-axis fold via PE transposes
    + DVE reduces. Host min-reduces the two halves of each batch.
  - Host: tiny [128,32]+[128,64] outputs per core -> final scalar in numpy.
"""

import numpy as np
import ml_dtypes

B = 4
NPTS = 8192          # pred points per batch
MPTS = 8192          # target points per batch
NH = NPTS // 2       # rows per core
P = 128              # partitions
FD = 512             # matmul free dim (one PSUM bank)
CFD = 2048           # cast-group free dim (4 banks)
R_TILES = NH // P    # 32 row tiles per core
N_G = MPTS // CFD    # 4 cast groups per row tile
N_C = MPTS // FD     # 16 col chunks
K_AUG = 16           # augmented contraction dim (hi/lo compensated bf16)
GPSIMD_COL_GS = ()   # cast groups whose col-path max runs on GPSIMD
                     # (this walrus rejects TensorTensor on Pool: NCC_IXCG966)

_CACHE = {}


def _split_multi_waits(bir_json):
    """This container's walrus caps sync waits at 1 per instruction. Split any
    instruction carrying N>1 waits into N-1 single-wait NoOps (same engine,
    inserted just before it) plus the original with one wait."""
    import json

    d = json.loads(bir_json)
    count = 0
    for fn in d["functions"]:
        for blk in fn["blocks"]:
            out = []
            for ins in blk["instructions"]:
                si = ins.get("sync_info")
                waits = (si or {}).get("on_wait") or []
                if len(waits) > 1:
                    for w in waits[:-1]:
                        count += 1
                        out.append({
                            "debug": ins.get("debug", 0),
                            "engine": ins["engine"],
                            "ins": [],
                            "outs": [],
                            "name": f"waitsplit-{count}",
                            "opcode": "NoOp",
                            "sync_info": {"on_update": [], "on_wait": [w]},
                        })
                    si["on_wait"] = [waits[-1]]
                out.append(ins)
            blk["instructions"] = out
    return json.dumps(d).encode()


def _patch_compiler():
    """Route bass2jax's walrus invocation through _split_multi_waits."""
    import concourse.bass2jax as b2j

    if getattr(b2j, "_waitsplit_patched", False):
        return
    orig = b2j.compile_bir_kernel

    def patched(bir_json, *args, **kwargs):
        return orig(_split_multi_waits(bir_json), *args, **kwargs)

    b2j.compile_bir_kernel = patched
    b2j._waitsplit_patched = True


def _build_program():
    import concourse.bass as bass
    import concourse.tile as tile
    from concourse import mybir
    from contextlib import ExitStack

    _patch_compiler()

    f32 = mybir.dt.float32
    bf16 = mybir.dt.bfloat16

    nc = bass.Bass("TRN2", target_bir_lowering=False, debug=False)

    predT_d = nc.dram_tensor("predT", [K_AUG, NH], bf16, kind="ExternalInput").ap()
    targT_d = nc.dram_tensor("targT", [K_AUG, MPTS], bf16, kind="ExternalInput").ap()
    ident_d = nc.dram_tensor("ident", [P, P], bf16, kind="ExternalInput").ap()
    chamx_d = nc.dram_tensor("chamx", [P, R_TILES], f32, kind="ExternalOutput").ap()
    chamy_d = nc.dram_tensor("chamy", [P, N_C * 4], f32, kind="ExternalOutput").ap()

    with tile.TileContext(nc) as tc, ExitStack() as ctx:
        const_pool = ctx.enter_context(tc.tile_pool(name="const", bufs=1))
        acc_pool = ctx.enter_context(tc.tile_pool(name="acc", bufs=1))
        cast_pool = ctx.enter_context(tc.tile_pool(name="cast", bufs=3))
        scr_pool = ctx.enter_context(tc.tile_pool(name="scr", bufs=5))

        predT_sb = const_pool.tile([K_AUG, NH], bf16)
        targT_sb = const_pool.tile([K_AUG, MPTS], bf16)
        ident_sb = const_pool.tile([P, P], bf16)
        # chunked loads so the first matmuls start as soon as their slice lands
        nc.sync.dma_start(predT_sb[:, :P], predT_d[:, :P])
        for g in range(N_G):
            sl = slice(g * CFD, (g + 1) * CFD)
            nc.sync.dma_start(targT_sb[:, sl], targT_d[:, sl])
        nc.sync.dma_start(predT_sb[:, P:], predT_d[:, P:])
        nc.sync.dma_start(ident_sb[:], ident_d[:])

        colacc = acc_pool.tile([P, MPTS], bf16)
        chamx_sb = acc_pool.tile([P, R_TILES], f32)
        chamy_sb = acc_pool.tile([P, N_C * 4], f32)

        with tc.tile_pool(name="mmpsum", bufs=2, space="PSUM") as mmp:
            for r in range(R_TILES):
                lhs = predT_sb[:, r * P:(r + 1) * P]
                # full-width bf16 image of this row tile's nd2
                cast_t = colacc if r == 0 else cast_pool.tile(
                    [P, MPTS], bf16, tag="cast"
                )
                for g in range(N_G):
                    pt = mmp.tile([P, CFD], f32, tag="mm")
                    for j in range(CFD // FD):
                        off = g * CFD + j * FD
                        nc.tensor.matmul(
                            pt[:, j * FD:(j + 1) * FD],
                            lhsT=lhs,
                            rhs=targT_sb[:, off:off + FD],
                            start=True,
                            stop=True,
                        )
                    # evacuate PSUM with a dtype cast on ACT
                    nc.scalar.copy(cast_t[:, g * CFD:(g + 1) * CFD], pt[:])
                # col path: one elementwise max accumulate per row tile
                if r > 0:
                    nc.vector.tensor_tensor(
                        out=colacc[:], in0=colacc[:], in1=cast_t[:],
                        op=mybir.AluOpType.max,
                    )
                # row path: pairwise fold tree 8192 -> 128, then one reduce
                prev = cast_t
                w = MPTS
                while w > 128:
                    nxt = scr_pool.tile([P, w // 2], bf16, tag=f"fold{w // 2}")
                    nc.vector.tensor_tensor(
                        out=nxt[:], in0=prev[:, :w // 2], in1=prev[:, w // 2:],
                        op=mybir.AluOpType.max,
                    )
                    prev = nxt
                    w //= 2
                nc.vector.tensor_reduce(
                    chamx_sb[:, r:r + 1], prev[:],
                    axis=mybir.AxisListType.X, op=mybir.AluOpType.max,
                )

        # partition-axis fold of colacc via PE transposes + free-dim reduces
        with tc.tile_pool(name="trpsum", bufs=4, space="PSUM") as trp:
            for c in range(N_C):
                tp = trp.tile([P, FD], bf16, tag="tr")
                for j in range(4):
                    src = colacc[:, c * FD + j * P: c * FD + (j + 1) * P]
                    nc.tensor.transpose(tp[:, j * P:(j + 1) * P], src, ident_sb[:])
                rr = tp[:].rearrange("p (j n) -> p j n", j=4)
                nc.vector.tensor_reduce(
                    chamy_sb[:, c * 4:(c + 1) * 4], rr,
                    axis=mybir.AxisListType.X, op=mybir.AluOpType.max,
                )

        nc.sync.dma_start(chamx_d[:], chamx_sb[:])
        nc.sync.dma_start(chamy_d[:], chamy_sb[:])

    return nc


def _augment(pred_b, target_b):
    """Hi/lo-compensated bf16 augmentation so a K=16 bf16 matmul reproduces
    nd2 = 2 p.t - |p|^2 - |t|^2 to ~1e-5 absolute despite bf16 inputs.

    pred_b/target_b: [npts, 3] fp32 -> lhsT [16, n], rhs [16, m] bf16."""
    bft = ml_dtypes.bfloat16

    def hilo(x):
        h = x.astype(bft).astype(np.float32)
        l = (x - h).astype(bft).astype(np.float32)
        return h, l

    p = np.asarray(pred_b, dtype=np.float32)
    t = np.asarray(target_b, dtype=np.float32)
    ph, pl = hilo(p)
    th, tl = hilo(t)
    p2h, p2l = hilo(np.sum(p * p, axis=1))
    t2h, t2l = hilo(np.sum(t * t, axis=1))
    n, m = p.shape[0], t.shape[0]
    L = np.zeros((K_AUG, n), np.float32)
    R = np.zeros((K_AUG, m), np.float32)
    L[0:3] = 2.0 * ph.T
    R[0:3] = th.T
    L[3:6] = 2.0 * ph.T
    R[3:6] = tl.T
    L[6:9] = 2.0 * pl.T
    R[6:9] = th.T
    L[9:12] = 2.0 * pl.T
    R[9:12] = tl.T
    L[12] = p2h
    R[12] = -1.0
    L[13] = p2l
    R[13] = -1.0
    L[14] = 1.0
    R[14] = -t2h
    L[15] = 1.0
    R[15] = -t2l
    return L.astype(bft), R.astype(bft)


def kernel(pred, target):
    from concourse.bass_utils import run_bass_kernel_spmd

    pred = np.asarray(pred, dtype=np.float32)
    target = np.asarray(target, dtype=np.float32)
    assert pred.shape == (B, NPTS, 3) and target.shape == (B, MPTS, 3)

    if "nc" not in _CACHE:
        _CACHE["nc"] = _build_program()
    nc = _CACHE["nc"]

    ident = np.eye(P, dtype=ml_dtypes.bfloat16)
    in_maps = []
    for core in range(8):
        b, h = core // 2, core % 2
        lhs, rhs = _augment(pred[b, h * NH:(h + 1) * NH], target[b])
        in_maps.append({"predT": lhs, "targT": rhs, "ident": ident})

    res = run_bass_kernel_spmd(nc, in_maps, list(range(8)))

    cham_x = np.empty((B, NPTS), dtype=np.float32)
    chamy_part = np.empty((B, 2, MPTS), dtype=np.float32)
    for core in range(8):
        b, h = core // 2, core % 2
        out_x = res.results[core]["chamx"]          # [128, 32] nd2 max
        out_y = res.results[core]["chamy"]          # [128, 64] nd2 max partial
        # n = r*128 + p
        cham_x[b, h * NH:(h + 1) * NH] = np.maximum(-out_x.T.reshape(NH), 0.0)
        # m = c*512 + j*128 + p
        chamy_part[b, h] = np.transpose(
            out_y.reshape(P, N_C, 4), (1, 2, 0)
        ).reshape(MPTS)
    cham_y = np.maximum(-np.max(chamy_part, axis=1), 0.0)

    loss = cham_x.mean(axis=1).mean() + cham_y.mean(axis=1).mean()
    return np.asarray(loss, dtype=np.float32)
